# revision 21
# baseline (speedup 1.0000x reference)
"""Causal single-head attention on 8 TRN2 NeuronCores.

Problem (hardcoded): B=16, T=2048, d_model=32, d_head=4, fp32 in/out.
  Q = q@Wq; K = q@Wk; V = q@Wv
  scores = Q K^T / sqrt(32), causal mask, softmax, out = W V.

Sharding: data-parallel over batch, 2 examples per core.

Per-core algorithm (per example):
  - qT [32,T] via PE transpose (fp16); QT/KT [4,T] and V-tiles via tiny matmuls.
  - Scores computed TRANSPOSED: S'[s,t] = K Q^T, so exp output E'[s,t] is
    directly the moving operand of the AV matmul with V' (V augmented with a
    ones column -> softmax denominator for free) as the stationary operand.
  - exp on ACT (the bottleneck engine: ~2.1M exps/example) reads wide PSUM
    chunks to amortize the ~352-cycle ACT overhead; only the causal lower
    triangle is ever computed.
  - Causal masking never touches E' in place (keeps each matmul operand
    single-writer, required by the 1-sync-wait-per-Matmult walrus limit):
    the AV matmul is split at the diagonal; the diagonal 128x128 block goes
    through a separate DVE-masked tile Em.
  - Normalization at the end (flash-style): transpose O'=[5,t-tile] blocks
    back with PE, then out = O'[:,0:4] * 1/O'[:,4] per partition on DVE.
  - All matmul operands fp16 (PSUM accumulation stays fp32); fp16 also makes
    tile_legalize split LDWEIGHTS from MATMUL, distributing sync waits.
"""

import numpy as np

import concourse.bass as bass
import concourse.mybir as mybir
import concourse.tile as tile
from concourse.masks import make_identity
from concourse.bass_utils import run_bass_kernel_spmd

B, T, D, H = 16, 2048, 32, 4
N_CORES = 8
EX_PER_CORE = B // N_CORES  # 2
SCALE = float(1.0 / np.sqrt(np.float32(D)))
FP = mybir.dt.float32
F16 = mybir.dt.float16
NT = T // 128   # 16 s-tiles per example
NR = T // 512   # 4  t-ranges per example
CHUNK = 1024    # exp/psum chunk width (2 PSUM banks)


def build_bass():
    nc = bass.Bass()
    q_in = nc.declare_dram_parameter("q_l", [EX_PER_CORE * T, D], FP, isOutput=False)
    wq_d = nc.declare_dram_parameter("Wq", [D, H], FP, isOutput=False)
    wk_d = nc.declare_dram_parameter("Wk", [D, H], FP, isOutput=False)
    wv_d = nc.declare_dram_parameter("Wv", [D, H], FP, isOutput=False)
    out_d = nc.declare_dram_parameter("out_l", [EX_PER_CORE * T, H], FP, isOutput=True)

    with tile.TileContext(nc) as tc:
        with (
            tc.tile_pool(name="const", bufs=1) as constp,
            tc.tile_pool(name="qsb", bufs=2) as qsbp,
            tc.tile_pool(name="qT", bufs=2) as qTp,
            tc.tile_pool(name="proj", bufs=2) as projp,
            tc.tile_pool(name="vp", bufs=2) as vpp,
            tc.tile_pool(name="E", bufs=3) as Ep,
            tc.tile_pool(name="Em", bufs=3) as Emp,
            tc.tile_pool(name="osb", bufs=2) as osbp,
            tc.tile_pool(name="of", bufs=4) as ofp,
            tc.tile_pool(name="psS", bufs=2, space="PSUM") as psS,
            tc.tile_pool(name="psO", bufs=1, space="PSUM") as psO,
        ):
            # --- constants ---
            # keep-mask for the diagonal block: 1 where t >= s else 0 (fp16)
            ut16 = constp.tile([128, 128], F16, tag="ut")
            nc.gpsimd.memset(ut16, 1.0)
            nc.gpsimd.affine_select(
                out=ut16, in_=ut16, compare_op=mybir.AluOpType.is_ge,
                fill=0.0, base=0, pattern=[[1, 128]], channel_multiplier=-1)
            id16 = constp.tile([128, 128], F16, tag="id")
            make_identity(nc, id16)
            wq_sb = constp.tile([D, H], FP, tag="wq")
            wk_sb = constp.tile([D, H], FP, tag="wk")
            wv_sb = constp.tile([D, H], FP, tag="wv")
            nc.sync.dma_start(out=wq_sb, in_=wq_d[:, :])
            nc.sync.dma_start(out=wk_sb, in_=wk_d[:, :])
            nc.sync.dma_start(out=wv_sb, in_=wv_d[:, :])
            wq16 = constp.tile([D, H], F16, tag="wq16")
            wk16 = constp.tile([D, H], F16, tag="wk16")
            wv16 = constp.tile([D, H], F16, tag="wv16")
            nc.vector.tensor_copy(wq16, wq_sb)
            nc.vector.tensor_copy(wk16, wk_sb)
            nc.vector.tensor_copy(wv16, wv_sb)

            o_all = ofp.tile([128, EX_PER_CORE, NT, H], FP, tag="oall")
            for ex in range(EX_PER_CORE):
                # ---- load q, cast to fp16, transpose to qT [32, T] ----
                q_ex = q_in[:, :][ex * T:(ex + 1) * T, :].rearrange(
                    "(n p) m -> p n m", p=128)
                q_sb = qsbp.tile([128, NT, D], FP, tag="q32")
                nc.sync.dma_start(out=q_sb, in_=q_ex)
                q16 = qsbp.tile([128, NT, D], F16, tag="q16")
                nc.vector.tensor_copy(q16, q_sb)
                qT_sb = qTp.tile([D, T], F16)
                for n in range(NT):
                    qT_ps = psS.tile([D, 128], F16, tag="S")
                    nc.tensor.transpose(qT_ps, q16[:, n, :], id16[:, 0:128])
                    nc.vector.tensor_copy(
                        qT_sb[:, n * 128:(n + 1) * 128], qT_ps)

                # ---- projections: QT/KT [4, T] fp16, V' [128, NT, 5] fp16 ----
                QT = projp.tile([H, T], F16, tag="QT")
                KT = projp.tile([H, T], F16, tag="KT")
                for r in range(NR):
                    sl = slice(r * 512, (r + 1) * 512)
                    pq = psS.tile([H, 512], FP, tag="S")
                    nc.tensor.matmul(pq, lhsT=wq16, rhs=qT_sb[:, sl],
                                     start=True, stop=True)
                    nc.vector.tensor_copy(QT[:, sl], pq)
                    pk = psS.tile([H, 512], FP, tag="S")
                    nc.tensor.matmul(pk, lhsT=wk16, rhs=qT_sb[:, sl],
                                     start=True, stop=True)
                    nc.vector.tensor_copy(KT[:, sl], pk)
                VP = vpp.tile([128, NT, 5], F16)
                nc.vector.memset(VP, 1.0)
                for n in range(NT):
                    pv = psS.tile([128, H], FP, tag="S")
                    nc.tensor.matmul(
                        pv, lhsT=qT_sb[:, n * 128:(n + 1) * 128], rhs=wv16,
                        start=True, stop=True)
                    nc.vector.tensor_copy(VP[:, n, 0:H], pv)

                # ---- main loop over s-tiles (software-pipelined) ----
                # PE executes its stream in order; emitting chunk c's AV
                # matmuls only after chunk c+1's score matmuls keeps PE busy
                # on scores while ACT runs exp(c) -> PE/ACT overlap instead
                # of ping-pong.
                outT_ps = psO.tile([8, T], FP)  # rows 0:5 used
                chunks = []
                for i in range(NT):
                    j0 = i // 4
                    base = j0 * 512          # global t of S-tile local col 0
                    Wd = T - base            # S-tile width
                    for off in range(0, Wd, CHUNK):
                        chunks.append((i, j0, base, off, min(CHUNK, Wd - off)))

                def emit_scores(c):
                    i, j0, base, off, cw = chunks[c]
                    pre = i * 128 - base
                    S = psS.tile([128, CHUNK], FP, tag="S")
                    for r in range(cw // 512):
                        nc.tensor.matmul(
                            S[:, r * 512:(r + 1) * 512],
                            lhsT=KT[:, i * 128:(i + 1) * 128],
                            rhs=QT[:, base + off + r * 512:
                                   base + off + (r + 1) * 512],
                            start=True, stop=True)
                    Et = Ep.tile([128, CHUNK], F16, tag="E")
                    lo = pre if off == 0 else 0  # skip all-masked prefix
                    nc.scalar.activation(
                        Et[:, lo:cw], S[:, lo:cw],
                        mybir.ActivationFunctionType.Exp, scale=SCALE)
                    if off == 0:
                        Em = Emp.tile([128, 128], F16, tag="Em")
                        nc.vector.tensor_mul(Em, Et[:, pre:pre + 128], ut16)
                    else:
                        Em = None
                    return Et, Em

                def emit_av(c, Et, Em):
                    i, j0, base, off, cw = chunks[c]
                    pre = i * 128 - base
                    for r in range(cw // 512):
                        jg = (base + off) // 512 + r    # global t-range
                        c0 = r * 512                    # local col
                        if jg == j0:
                            nc.tensor.matmul(
                                outT_ps[0:5, i * 128:(i + 1) * 128],
                                lhsT=VP[:, i, :], rhs=Em,
                                start=(i == 0), stop=True,
                                skip_group_check=True)
                            if pre + 128 < 512:
                                nc.tensor.matmul(
                                    outT_ps[0:5, (i + 1) * 128:
                                            (jg + 1) * 512],
                                    lhsT=VP[:, i, :],
                                    rhs=Et[:, pre + 128:512],
                                    start=(i == 0), stop=False,
                                    skip_group_check=True)
                        else:
                            nc.tensor.matmul(
                                outT_ps[0:5, jg * 512:(jg + 1) * 512],
                                lhsT=VP[:, i, :],
                                rhs=Et[:, c0:c0 + 512],
                                start=(i == 0), stop=(i == 4 * jg + 3),
                                skip_group_check=True)

                prev = None
                for c in range(len(chunks)):
                    cur = emit_scores(c)
                    if prev is not None:
                        emit_av(c - 1, *prev)
                    prev = cur
                emit_av(len(chunks) - 1, *prev)

                # ---- epilogue: transpose back, normalize, store ----
                oT16 = osbp.tile([5, T], F16)
                nc.vector.tensor_copy(oT16, outT_ps[0:5, :])
                for n in range(NT):
                    o_ps = psS.tile([128, 5], F16, tag="S")
                    nc.tensor.transpose(
                        o_ps, oT16[:, n * 128:(n + 1) * 128], id16[0:5, 0:5])
                    rinv = ofp.tile([128, 1], FP, tag="rinv")
                    nc.vector.reciprocal(rinv, o_ps[:, 4:5])
                    nc.vector.tensor_scalar_mul(
                        o_all[:, ex, n, :], o_ps[:, 0:H], rinv)
            nc.sync.dma_start(
                out=out_d[:, :].rearrange("(e n p) h -> p e n h", p=128, n=NT),
                in_=o_all)
    return nc


def reduce_waits(nc):
    """Transitively-redundant sync-wait elimination.

    This walrus build rejects instructions with more sync waits than their
    lowered struct has slots (e.g. 1 for Matmult). Tile's wait assignment is
    per-proc minimal but not transitive, and emits redundant same-engine
    self-waits. We drop every wait already implied by happens-before:
      - engines dispatch in order, and sem waits gate dispatch, so an
        instruction inherits its proc's dispatch-time knowledge;
      - a single-proc semaphore reaching value v implies the dispatch
        knowledge of its v-th incrementer (increments complete in order);
      - multi-proc semaphores (barriers) only convey knowledge when waiting
        for the full increment count.
    """
    from collections import defaultdict

    sem_updaters = defaultdict(set)   # sem -> set of proc names
    bad_sems = set()                  # sems with non-monotone updates
    insts = []
    for f in nc.m.functions:
        for blk in f.blocks:
            for inst in blk.instructions:
                si = getattr(inst, "sync_info", None)
                if si is None:
                    continue
                proc = str(getattr(inst, "engine", "?"))
                # DMA waits are evaluated by the DGE queue when it processes
                # the descriptor, not by the issuing engine: the queue (keyed
                # by its completion sem) is the proc, and the issuing
                # engine's clock must not absorb the DMA's waits.
                if inst.__class__.__name__ == "InstDMACopy":
                    upd = [u.ant_name for u in si.on_update]
                    proc = "Q:" + (upd[0] if upd else inst.name)
                insts.append((inst, si, proc))
                for u in si.on_update:
                    sem_updaters[u.ant_name].add(proc)
                    if u.update_reg is not None or u.update_mode not in (
                            "sem-inc", "sem-add-imm"):
                        bad_sems.add(u.ant_name)

    proc_clock = defaultdict(dict)    # proc -> {sem: value known}
    sem_count = defaultdict(int)      # sem -> stream increment total
    sem_vc = defaultdict(dict)        # sem -> {value: knowledge dict}
    ndrop = 0
    for inst, si, proc in insts:
        know0 = proc_clock[proc]

        def wait_vc(w):
            """Knowledge implied by wait w being satisfied (dict)."""
            s, v = w.ant_name, w.wait_value
            if s in bad_sems or w.wait_reg is not None or \
                    w.wait_mode != "sem-ge-imm":
                return None  # unanalyzable: keep, conveys nothing
            k = {s: v}
            vc = sem_vc[s].get(v)
            if vc is not None and (
                len(sem_updaters[s]) <= 1 or v >= sem_count[s]
            ):
                for ks, kv in vc.items():
                    if k.get(ks, -1) < kv:
                        k[ks] = kv
            return k

        def implied_by(w, k):
            s, v = w.ant_name, w.wait_value
            if s in bad_sems or w.wait_reg is not None or \
                    w.wait_mode != "sem-ge-imm":
                return False
            # self-wait: sem only incremented by this proc, which completes
            # in order (PE matmuls are pc-monotone in start and end; DVE/ACT
            # drain per op) -> implied by program order. NOT valid for
            # GPSIMD (8 concurrent Q7 queues) or DMA-completion sems.
            import os
            selfdrop = os.environ.get("SELFDROP", "").split(",")
            if (sem_updaters[s] == {proc}
                    and proc in tuple("EngineType." + e for e in selfdrop if e)
                    and v <= sem_count[s]):
                return True
            return k.get(s, -1) >= v

        vcs = {id(w): wait_vc(w) for w in si.on_wait}
        kept = list(si.on_wait)
        for w in list(kept):
            k = dict(know0)
            for o in kept:
                if o is w:
                    continue
                ovc = vcs[id(o)]
                if ovc:
                    for ks, kv in ovc.items():
                        if k.get(ks, -1) < kv:
                            k[ks] = kv
            if implied_by(w, k):
                kept.remove(w)
                ndrop += 1
        if len(kept) != len(si.on_wait):
            si.on_wait = kept
            inst.sync_info = si
        # dispatch knowledge: prior clock + all original waits (kept or
        # implied, both hold at dispatch)
        know = dict(know0)
        for wid, vc in vcs.items():
            if vc:
                for ks, kv in vc.items():
                    if know.get(ks, -1) < kv:
                        know[ks] = kv
        proc_clock[proc] = know
        for u in si.on_update:
            if u.update_reg is not None or u.update_mode not in (
                    "sem-inc", "sem-add-imm"):
                continue
            s = u.ant_name
            if s in bad_sems:
                continue
            sem_count[s] += u.update_value
            v = sem_count[s]
            if len(sem_updaters[s]) <= 1:
                vc = dict(know)
            else:
                # unordered increments: union knowledge (valid only for
                # wait-for-all, enforced at the waiter above)
                vc = dict(sem_vc[s].get(max(sem_vc[s], default=0), {}))
                for ks, kv in know.items():
                    if vc.get(ks, -1) < kv:
                        vc[ks] = kv
            vc[s] = v
            sem_vc[s][v] = vc
    return ndrop


_NC = None


def _get_nc():
    global _NC
    if _NC is None:
        _NC = build_bass()
        reduce_waits(_NC)
    return _NC


def _run(q, Wq, Wk, Wv, **kw):
    nc = _get_nc()
    q = np.ascontiguousarray(np.asarray(q, dtype=np.float32))
    in_maps = []
    for c in range(N_CORES):
        in_maps.append({
            "q_l": np.ascontiguousarray(
                q[EX_PER_CORE * c: EX_PER_CORE * (c + 1)].reshape(
                    EX_PER_CORE * T, D)),
            "Wq": np.ascontiguousarray(np.asarray(Wq, dtype=np.float32)),
            "Wk": np.ascontiguousarray(np.asarray(Wk, dtype=np.float32)),
            "Wv": np.ascontiguousarray(np.asarray(Wv, dtype=np.float32)),
        })
    res = run_bass_kernel_spmd(nc, in_maps, list(range(N_CORES)), **kw)
    out = np.stack([
        np.asarray(res.results[c]["out_l"]).reshape(EX_PER_CORE, T, H)
        for c in range(N_CORES)
    ]).reshape(B, T, H)
    return out, res


def kernel(q, Wq, Wk, Wv):
    out, _ = _run(q, Wq, Wk, Wv)
    return out
